# revision 2
# baseline (speedup 1.0000x reference)
"""Trainium2 Bass kernel v2 for nn_MoEConnectionProcessor.

Two-phase design per core (BS=32768 cells):
  Phase A (per 512-cell macro): DMA nbN (natural bf16) + nbT1 (host-staged
  block-T of m1*nb, valid since m*tanh(x)=tanh(m*x) for binary m and zero
  b_msg); DVE products with count-folded masks (dup2 packing for the 2x DVE
  mode); PE identity-ksum over [p0|p2|nbN] (26 accumulating matmuls, 384-col
  strided moving); PE msgs matmul (block-diag kron(I4,W_msg)) + Scalar tanh;
  DVE strided tensor_reduce over k for the functional aggregation.
  Sums land in a per-supermacro f32 accumulator laid out per subtile as
  [mean_loc | mean_dis | S0 | aggT_raw(block-T)].

  Phase B (per 8192-cell supermacro): ONE bf16 cast + ONE stream-transpose
  converts all four slot groups (natural->block-T and the block-T agg ->
  natural simultaneously); chunked block-diag expert matmuls + single-table
  activations (sigmoid via tanh(0.5x) identity, softmax exp, relu); gating
  dots; weighted combine. All per-cell ops run 64 subtiles wide.
"""

import numpy as np
import ml_dtypes
from contextlib import ExitStack

import concourse.bass as bass
import concourse.bacc as bacc
import concourse.tile as tile
import concourse.mybir as mybir

B, K, D = 262144, 26, 32
N_CORES = 8
BS = B // N_CORES            # 32768 cells per core
SUB = BS // 128              # 256 subtiles
AM_SUB = 4                   # subtiles per A-macro (512 cells)
NA = SUB // AM_SUB           # 64 A-macros
SM_SUB = 32                  # subtiles per supermacro (4096 cells)
NS = SUB // SM_SUB           # 4 supermacros
AM_PER_SM = SM_SUB // AM_SUB  # 16
FR = K * D                   # 832 cols per subtile of neighbor data
AFR = AM_SUB * FR            # 3328 per A-macro
DT_STEP = 1.0 / 3.0

dt = mybir.dt
bf16 = ml_dtypes.bfloat16
AF = mybir.ActivationFunctionType
ALU = mybir.AluOpType

_WSLOTS = ["W4msg", "Wl_t", "Wl_b", "Wu_t", "Wu_b", "Wc_t", "Wc_b",
           "Wc_dt", "Wg1_t", "Wg1_b", "I128"]
WC_COLS = 128 * len(_WSLOTS) + 96
# smA: per A-macro [m0w-dup2 (4*52) | m2w-dup2 (4*52)] = 416 cols
SMA_COLS = AM_SUB * 52 * 2
# smB: per supermacro [scl1-dup2 (128) | csn (2048) | csT (2048)]
SMB_COLS = SM_SUB * 2 + SM_SUB * 32 * 2


def _wslot(name):
    return 128 * _WSLOTS.index(name)


def build_program():
    nc = bacc.Bacc("TRN2", target_bir_lowering=False, debug=False,
                   num_devices=N_CORES)
    a_nbn = nc.dram_tensor("nbn", [128, NA * AFR], dt.bfloat16, kind="ExternalInput").ap()
    a_nbt = nc.dram_tensor("nbt", [128, NA * AFR], dt.bfloat16, kind="ExternalInput").ap()
    a_smA = nc.dram_tensor("smA", [128, NA * SMA_COLS], dt.bfloat16, kind="ExternalInput").ap()
    a_smB = nc.dram_tensor("smB", [128, NS * SMB_COLS], dt.bfloat16, kind="ExternalInput").ap()
    a_wc = nc.dram_tensor("wc", [128, WC_COLS], dt.bfloat16, kind="ExternalInput").ap()
    a_bc = nc.dram_tensor("bc", [128, 8], dt.float32, kind="ExternalInput").ap()
    a_out = nc.dram_tensor("out", [128, SUB * D], dt.float32, kind="ExternalOutput").ap()

    with tile.TileContext(nc) as tc:
        _body(tc, a_nbn, a_nbt, a_smA, a_smB, a_wc, a_bc, a_out)
    nc.compile()
    return nc


def _body(tc, a_nbn, a_nbt, a_smA, a_smB, a_wc, a_bc, a_out):
    nc = tc.nc
    with ExitStack() as ctx:
        cpool = ctx.enter_context(tc.tile_pool(name="const", bufs=1))
        pA = ctx.enter_context(tc.tile_pool(name="pA", bufs=3))
        pS = ctx.enter_context(tc.tile_pool(name="pS", bufs=1))
        pSm = ctx.enter_context(tc.tile_pool(name="pSm", bufs=2))
        pB = ctx.enter_context(tc.tile_pool(name="pB", bufs=1))
        pps_m = ctx.enter_context(tc.tile_pool(name="psm", bufs=2, space="PSUM"))
        pps_k = ctx.enter_context(tc.tile_pool(name="psk", bufs=2, space="PSUM"))
        pps_b = ctx.enter_context(tc.tile_pool(name="psb", bufs=2, space="PSUM"))

        wc = cpool.tile([128, WC_COLS], dt.bfloat16, tag="wc")
        nc.sync.dma_start(wc[:], a_wc)
        bc = cpool.tile([128, 8], dt.float32, tag="bc")
        nc.sync.dma_start(bc[:], a_bc)

        def W(name):
            return wc[:, _wslot(name): _wslot(name) + 128]

        w2rep = wc[:, 128 * len(_WSLOTS): 128 * len(_WSLOTS) + 96]
        b_loc = bc[:, 0:1]
        b_updh = bc[:, 1:2]   # 0.5 * b_upd
        b_cnf = bc[:, 2:3]
        b_g1 = bc[:, 3:4]

        for sm in range(NS):
            smB = pSm.tile([128, SMB_COLS], dt.bfloat16, tag="smB")
            nc.sync.dma_start(smB[:], a_smB[:, sm * SMB_COLS:(sm + 1) * SMB_COLS])
            scl1d = smB[:, 0:SM_SUB * 2]                     # [p, (s,2)] dup2
            NB = SM_SUB * D
            csn = smB[:, SM_SUB * 2: SM_SUB * 2 + NB]        # [p, (s d)]
            csT = smB[:, SM_SUB * 2 + NB: SM_SUB * 2 + 2 * NB]

            # bf16 sums accumulator, per subtile [ml | md | S0 | aggT_raw]
            sums16 = pS.tile([128, SM_SUB * 128], dt.bfloat16, tag="sums16")

            # ================= PHASE A =================
            for am in range(AM_PER_SM):
                M = sm * AM_PER_SM + am
                P = pA.tile([128, 3 * AFR], dt.bfloat16, tag="P")
                nc.sync.dma_start(P[:, 2 * AFR:3 * AFR],
                                  a_nbn[:, M * AFR:(M + 1) * AFR])
                nbt = pA.tile([128, AFR], dt.bfloat16, tag="nbt")
                nc.sync.dma_start(nbt[:], a_nbt[:, M * AFR:(M + 1) * AFR])
                mk = pA.tile([128, SMA_COLS], dt.bfloat16, tag="mk")
                nc.sync.dma_start(mk[:], a_smA[:, M * SMA_COLS:(M + 1) * SMA_COLS])

                # masked products (dup2 views, 2x mode)
                nbv = P[:, 2 * AFR:3 * AFR].rearrange(
                    "p (tk s two) -> p tk s two", tk=AM_SUB * K, s=16)
                for seg in (0, 1):
                    mview = mk[:, seg * AM_SUB * 52:(seg + 1) * AM_SUB * 52]\
                        .rearrange("p (tk two) -> p tk two", tk=AM_SUB * K)\
                        .unsqueeze(2).to_broadcast((128, AM_SUB * K, 16, 2))
                    nc.vector.tensor_tensor(
                        out=P[:, seg * AFR:(seg + 1) * AFR].rearrange(
                            "p (tk s two) -> p tk s two", tk=AM_SUB * K, s=16),
                        in0=nbv, in1=mview, op=ALU.mult)

                # msgs matmuls + tanh (per subtile t)
                tanhT = pA.tile([128, AFR], dt.bfloat16, tag="tanhT")
                for t in range(AM_SUB):
                    ps = pps_m.tile([128, 1024], dt.float32, tag="msg")
                    src = nbt[:, t * FR:(t + 1) * FR]
                    nc.tensor.matmul(ps[:, 0:416], W("W4msg"), src[:, 0:416],
                                     start=True, stop=True)
                    nc.tensor.matmul(ps[:, 512:928], W("W4msg"), src[:, 416:832],
                                     start=True, stop=True)
                    nc.scalar.activation(
                        tanhT[:, t * FR:(t + 1) * FR].rearrange(
                            "p (c h k) -> p h c k", c=32, h=2),
                        ps[:].rearrange("p (h k c) -> p h c k", h=2, k=16)
                            [:, :, :, 0:13],
                        AF.Tanh)

                # ksum on PE: 26 accumulating matmuls over [p0|p2|nbN]
                psk = pps_k.tile([128, 512], dt.float32, tag="ksum")
                Pv = P[:].rearrange("p (st k d) -> p k st d", st=12, k=K)
                for k in range(K):
                    nc.tensor.matmul(psk[:, 0:384], W("I128"), Pv[:, k],
                                     start=(k == 0), stop=(k == K - 1))

                base = am * AM_SUB * 128
                sview = sums16[:, base:base + AM_SUB * 128].rearrange(
                    "p (t sl d) -> p sl t d", t=AM_SUB, sl=4)
                nc.scalar.activation(
                    sview[:, 0:3],
                    psk[:, 0:384].rearrange("p (s t d) -> p s t d", s=3, t=AM_SUB),
                    AF.Copy)

                # functional aggregation: unmasked k-reduce of tanhT
                # (k packed innermost; DVE ALUs accumulate internally at f32)
                with nc.allow_low_precision(reason="k-reduce rounds once at out"):
                    nc.vector.tensor_reduce(
                        out=sview[:, 3],
                        in_=tanhT[:].rearrange("p (t c k) -> p t c k",
                                               t=AM_SUB, c=32),
                        axis=mybir.AxisListType.X, op=ALU.add)

            # ================= PHASE B =================
            sumsT = pS.tile([128, SM_SUB * 128], dt.bfloat16, tag="sumsT")
            nc.vector.transpose(sumsT[:], sums16[:])
            # sumsT per subtile: [mlT | mdT | mnT | agg_nat_raw]
            agg = pB.tile([128, NB], dt.bfloat16, tag="agg")
            nc.vector.tensor_tensor(
                out=agg[:].rearrange("p (s h two) -> p s h two", s=SM_SUB, h=16),
                in0=sumsT[:].rearrange("p (s c) -> p s c", c=128)[:, :, 96:128]
                    .rearrange("p s (h two) -> p s h two", h=16),
                in1=scl1d.rearrange("p (s two) -> p s two", s=SM_SUB)
                    .unsqueeze(2).to_broadcast((128, SM_SUB, 16, 2)),
                op=ALU.mult)
            aggT = pB.tile([128, NB], dt.bfloat16, tag="aggT")
            nc.vector.transpose(aggT[:], agg[:])
            tagg = pB.tile([128, NB], dt.bfloat16, tag="tagg")
            nc.scalar.activation(tagg[:], agg[:], AF.Tanh)

            localT = pB.tile([128, NB], dt.bfloat16, tag="localT")
            uT = pB.tile([128, NB], dt.bfloat16, tag="uT")
            hT = pB.tile([128, NB], dt.bfloat16, tag="hT")
            distT = pB.tile([128, NB], dt.bfloat16, tag="distT")

            NCH = 2
            CW = NB // NCH        # 512 cols per chunk
            CS = SM_SUB // NCH    # 16 subtiles per chunk
            for ch in range(NCH):
                cs_c = csT[:, ch * CW:(ch + 1) * CW]
                sv = sumsT[:].rearrange("p (s c) -> p s c", c=128)[
                    :, ch * CS:(ch + 1) * CS]
                mlT_c = sv[:, :, 0:32]
                mdT_c = sv[:, :, 32:64]
                mnT_c = sv[:, :, 64:96]
                aggT_c = aggT[:, ch * CW:(ch + 1) * CW]

                ps1 = pps_b.tile([128, 512], dt.float32, tag="psb")
                nc.tensor.matmul(ps1[:], W("Wl_t"), cs_c, start=True, stop=False)
                nc.tensor.matmul(ps1[:], W("Wl_b"), mlT_c, start=False, stop=True)
                nc.scalar.activation(localT[:, ch * CW:(ch + 1) * CW], ps1[:],
                                     AF.Tanh, bias=b_loc)

                ps2 = pps_b.tile([128, 512], dt.float32, tag="psb")
                nc.tensor.matmul(ps2[:], W("Wu_t"), cs_c, start=True, stop=False)
                nc.tensor.matmul(ps2[:], W("Wu_b"), aggT_c, start=False, stop=True)
                nc.scalar.activation(uT[:, ch * CW:(ch + 1) * CW], ps2[:],
                                     AF.Tanh, bias=b_updh, scale=0.5)

                ps3 = pps_b.tile([128, 512], dt.float32, tag="psb")
                nc.tensor.matmul(ps3[:], W("Wg1_t"), cs_c, start=True, stop=False)
                nc.tensor.matmul(ps3[:], W("Wg1_b"), mnT_c, start=False, stop=True)
                nc.scalar.activation(hT[:, ch * CW:(ch + 1) * CW], ps3[:],
                                     AF.Relu, bias=b_g1)

                # CNF Euler chain, x-updates folded into PE accumulation:
                # v_{s+1} uses Wc_t*cs + dt*Wc_t*(v1+..+vs) + Wc_b*md
                vbs = []
                for s in range(3):
                    ps4 = pps_b.tile([128, 512], dt.float32, tag="psb")
                    nc.tensor.matmul(ps4[:], W("Wc_t"), cs_c, start=True, stop=False)
                    for vprev in vbs:
                        nc.tensor.matmul(ps4[:], W("Wc_dt"), vprev[:],
                                         start=False, stop=False)
                    nc.tensor.matmul(ps4[:], W("Wc_b"), mdT_c, start=False, stop=True)
                    vb = pB.tile([128, 512], dt.bfloat16, tag=f"vb{s}")
                    nc.scalar.activation(vb[:], ps4[:], AF.Tanh, bias=b_cnf)
                    vbs.append(vb)
                v12 = pB.tile([128, 512], dt.bfloat16, tag="v12")
                nc.vector.tensor_tensor(out=v12[:], in0=vbs[0][:], in1=vbs[1][:],
                                        op=ALU.add)
                v123 = pB.tile([128, 512], dt.bfloat16, tag="v123")
                nc.vector.tensor_tensor(out=v123[:], in0=v12[:], in1=vbs[2][:],
                                        op=ALU.add)
                nc.vector.scalar_tensor_tensor(
                    out=distT[:, ch * CW:(ch + 1) * CW], in0=v123[:],
                    scalar=DT_STEP, in1=cs_c, op0=ALU.mult, op1=ALU.add)

            # transposes to natural
            u_nat = pB.tile([128, NB], dt.bfloat16, tag="u_nat")
            nc.vector.transpose(u_nat[:], uT[:])
            h_nat = pB.tile([128, NB], dt.bfloat16, tag="h_nat")
            nc.vector.transpose(h_nat[:], hT[:])
            loc_nat = pB.tile([128, NB], dt.bfloat16, tag="loc_nat")
            nc.vector.transpose(loc_nat[:], localT[:])
            dist_nat = pB.tile([128, NB], dt.bfloat16, tag="dist_nat")
            nc.vector.transpose(dist_nat[:], distT[:])

            # gating
            lg = pB.tile([128, 3 * SM_SUB], dt.float32, tag="lg")
            for g in range(3):
                gp = pB.tile([128, NB], dt.bfloat16, tag=f"gp{g}")
                nc.gpsimd.tensor_tensor(
                    out=gp[:], in0=h_nat[:],
                    in1=w2rep[:, 32 * g:32 * g + 32].unsqueeze(1)
                        .to_broadcast((128, SM_SUB, D)),
                    op=ALU.mult)
                nc.vector.tensor_reduce(
                    out=lg[:, g * SM_SUB:(g + 1) * SM_SUB],
                    in_=gp[:].rearrange("p (s d) -> p s d", s=SM_SUB),
                    axis=mybir.AxisListType.X, op=ALU.add)
            eg = pB.tile([128, 3 * SM_SUB], dt.float32, tag="eg")
            nc.scalar.activation(eg[:], lg[:], AF.Exp)
            se = pB.tile([128, SM_SUB], dt.float32, tag="se")
            nc.vector.tensor_reduce(
                out=se[:], in_=eg[:].rearrange("p (g s) -> p s g", g=3),
                axis=mybir.AxisListType.X, op=ALU.add)
            rinv = pB.tile([128, SM_SUB], dt.float32, tag="rinv")
            nc.vector.reciprocal(rinv[:], se[:])
            gts2 = pB.tile([128, 6 * SM_SUB], dt.bfloat16, tag="gts2")
            nc.vector.tensor_tensor(
                out=gts2[:].rearrange("p (g s two) -> p g s two", g=3, two=2),
                in0=eg[:].rearrange("p (g s) -> p g s", g=3)
                    .unsqueeze(3).to_broadcast((128, 3, SM_SUB, 2)),
                in1=rinv[:].unsqueeze(1).unsqueeze(3)
                    .to_broadcast((128, 3, SM_SUB, 2)),
                op=ALU.mult)

            # func expert (natural)
            d2 = pB.tile([128, NB], dt.bfloat16, tag="d2")
            nc.vector.tensor_tensor(out=d2[:], in0=tagg[:], in1=csn, op=ALU.subtract)
            h1 = pB.tile([128, NB], dt.bfloat16, tag="h1")
            nc.vector.scalar_tensor_tensor(out=h1[:], in0=d2[:], scalar=0.5,
                                           in1=csn, op0=ALU.mult, op1=ALU.add)
            f1 = pB.tile([128, NB], dt.bfloat16, tag="f1")
            nc.gpsimd.tensor_tensor(out=f1[:], in0=u_nat[:], in1=d2[:], op=ALU.mult)
            fn = pB.tile([128, NB], dt.bfloat16, tag="fn")
            nc.vector.scalar_tensor_tensor(out=fn[:], in0=f1[:], scalar=0.5,
                                           in1=h1[:], op0=ALU.mult, op1=ALU.add)

            # weighted combine
            def gv(g):
                return gts2[:, g * 2 * SM_SUB:(g + 1) * 2 * SM_SUB].rearrange(
                    "p (s two) -> p s two", s=SM_SUB).unsqueeze(2)\
                    .to_broadcast((128, SM_SUB, 16, 2))

            def ev(t_):
                return t_.rearrange("p (s h two) -> p s h two", s=SM_SUB, h=16)

            t0 = pB.tile([128, NB], dt.bfloat16, tag="t0")
            nc.gpsimd.tensor_tensor(out=ev(t0[:]), in0=ev(loc_nat[:]), in1=gv(0),
                                    op=ALU.mult)
            t1 = pB.tile([128, NB], dt.bfloat16, tag="t1")
            nc.vector.tensor_tensor(out=ev(t1[:]), in0=ev(fn[:]), in1=gv(1),
                                    op=ALU.mult)
            t2 = pB.tile([128, NB], dt.bfloat16, tag="t2")
            nc.gpsimd.tensor_tensor(out=ev(t2[:]), in0=ev(dist_nat[:]), in1=gv(2),
                                    op=ALU.mult)
            a1 = pB.tile([128, NB], dt.float32, tag="a1")
            nc.gpsimd.tensor_tensor(out=a1[:], in0=t0[:], in1=t1[:], op=ALU.add)
            acc = pB.tile([128, NB], dt.float32, tag="acc")
            nc.vector.tensor_tensor(out=acc[:], in0=a1[:], in1=t2[:], op=ALU.add)

            nc.sync.dma_start(a_out[:, sm * NB:(sm + 1) * NB], acc[:])


# ---------------------------------------------------------------------------
# host staging
# ---------------------------------------------------------------------------

def stage_weights(inputs):
    f32 = np.float32
    W_local = np.asarray(inputs["W_local"], f32)
    W_msg = np.asarray(inputs["W_msg"], f32)
    W_upd = np.asarray(inputs["W_upd"], f32)
    W_cnf = np.asarray(inputs["W_cnf"], f32)
    W_g1 = np.asarray(inputs["W_g1"], f32)
    W_g2 = np.asarray(inputs["W_g2"], f32)

    assert np.abs(np.asarray(inputs["b_msg"])).max() == 0.0, \
        "nonzero b_msg needs the host correction term (not wired)"
    assert np.abs(np.asarray(inputs["b_g2"])).max() == 0.0, \
        "nonzero b_g2 needs the exp-fold term (not wired)"

    eye4 = np.eye(4, dtype=f32)

    def kron4(w):
        return np.kron(eye4, w)

    wparts = {
        "W4msg": kron4(W_msg),
        "Wl_t": kron4(W_local[:D]), "Wl_b": kron4(W_local[D:]),
        "Wu_t": kron4(W_upd[:D]), "Wu_b": kron4(W_upd[D:]),
        "Wc_t": kron4(W_cnf[:D]), "Wc_b": kron4(W_cnf[D:]),
        "Wc_dt": kron4(W_cnf[:D]) * DT_STEP,
        "Wg1_t": kron4(W_g1[:D]), "Wg1_b": kron4(W_g1[D:] / K),
        "I128": np.eye(128, dtype=f32),
    }
    wcq = np.zeros((128, WC_COLS), f32)
    for name in _WSLOTS:
        wcq[:, _wslot(name):_wslot(name) + 128] = wparts[name]
    for g in range(3):
        wcq[:, 128 * len(_WSLOTS) + 32 * g:
            128 * len(_WSLOTS) + 32 * g + 32] = W_g2[:, g][None, :]
    wcq = wcq.astype(bf16)

    bcq = np.zeros((128, 8), f32)
    bcq[:, 0] = np.tile(np.asarray(inputs["b_local"], f32), 4)
    bcq[:, 1] = np.tile(np.asarray(inputs["b_upd"], f32) * 0.5, 4)
    bcq[:, 2] = np.tile(np.asarray(inputs["b_cnf"], f32), 4)
    bcq[:, 3] = np.tile(np.asarray(inputs["b_g1"], f32), 4)
    return wcq, bcq


def stage_core(nb_c, cs_c, tr_c):
    """Builds per-core input dict (nb_c [BS,K,D] f32, cs_c [BS,D], tr_c [BS,K])."""
    f32 = np.float32
    m0 = (tr_c == 0)
    m1 = (tr_c == 1)
    m2 = (tr_c == 2)
    cnt0 = np.maximum(m0.sum(-1), 1.0).astype(f32)
    cnt1 = np.maximum(m1.sum(-1), 1.0).astype(f32)
    cnt2 = np.maximum(m2.sum(-1), 1.0).astype(f32)
    m0w = (m0.astype(f32) / cnt0[:, None]).astype(bf16)
    m2w = (m2.astype(f32) / cnt2[:, None]).astype(bf16)
    scl1 = (1.0 / cnt1).astype(bf16)

    nb16 = nb_c.astype(bf16)
    # nbn natural: [M, t, p, k, d] -> [p, M*(t*832 + k*32 + d)]
    nbn = np.ascontiguousarray(
        nb16.reshape(NA, AM_SUB, 128, K, D).transpose(2, 0, 1, 3, 4)
    ).reshape(128, NA * AFR)

    # nbt1 block-T of m1*nb: [M,t,a,c,k,d] -> [a,d | M,t,k,c]
    m1nb = (nb_c * m1[:, :, None]).astype(bf16)
    nbt1 = np.ascontiguousarray(
        m1nb.reshape(NA, AM_SUB, 4, 32, K, D).transpose(2, 5, 0, 1, 4, 3)
    ).reshape(128, NA * AFR)

    # masks dup2 per subtile: [s, p, k, 2] -> [p, s*52]
    def dup2(m):
        md = np.repeat(m.reshape(SUB, 128, K), 2, axis=2)  # [s, p, 2K]
        return np.ascontiguousarray(md.transpose(1, 0, 2)).reshape(128, SUB * 52)

    m0d = dup2(m0w)
    m2d = dup2(m2w)
    smA = np.zeros((128, NA * SMA_COLS), bf16)
    for M in range(NA):
        s0, s1 = M * AM_SUB * 52, (M + 1) * AM_SUB * 52
        smA[:, M * SMA_COLS: M * SMA_COLS + AM_SUB * 52] = m0d[:, s0:s1]
        smA[:, M * SMA_COLS + AM_SUB * 52:(M + 1) * SMA_COLS] = m2d[:, s0:s1]

    # smB: [scl1-dup2 | csn | csT] per supermacro
    cs16 = cs_c.astype(bf16)
    csn_full = np.ascontiguousarray(
        cs16.reshape(SUB, 128, D).transpose(1, 0, 2)).reshape(128, SUB * D)
    csT_full = np.ascontiguousarray(
        cs16.reshape(SUB, 4, 32, D).transpose(1, 3, 0, 2)).reshape(128, SUB * D)
    scl1d = np.repeat(scl1.reshape(SUB, 128, 1), 2, axis=2)  # [s, p, 2]
    scl1d = np.ascontiguousarray(scl1d.transpose(1, 0, 2)).reshape(128, SUB * 2)
    NB = SM_SUB * D
    smB = np.zeros((128, NS * SMB_COLS), bf16)
    for sm in range(NS):
        o = sm * SMB_COLS
        smB[:, o:o + SM_SUB * 2] = scl1d[:, sm * SM_SUB * 2:(sm + 1) * SM_SUB * 2]
        smB[:, o + SM_SUB * 2: o + SM_SUB * 2 + NB] = \
            csn_full[:, sm * NB:(sm + 1) * NB]
        smB[:, o + SM_SUB * 2 + NB: o + SMB_COLS] = \
            csT_full[:, sm * NB:(sm + 1) * NB]
    return {"nbn": nbn, "nbt": nbt1, "smA": smA, "smB": smB}


def stage_inputs(inputs):
    cs = np.asarray(inputs["current_state"], np.float32)
    nb = np.asarray(inputs["neighbor_states"], np.float32)
    tiers = np.asarray(inputs["tier_ids"], np.int32)
    wcq, bcq = stage_weights(inputs)
    in_maps = []
    for c in range(N_CORES):
        rs = slice(c * BS, (c + 1) * BS)
        m = stage_core(nb[rs], cs[rs], tiers[rs])
        m["wc"] = wcq
        m["bc"] = bcq
        in_maps.append(m)
    return in_maps


def unpack_out(res_out):
    # [128, SUB*32] -> [BS, 32]
    return np.ascontiguousarray(
        res_out.reshape(128, SUB, D).transpose(1, 0, 2)).reshape(BS, D)


_PROGRAM_CACHE = {}


def kernel(**inputs):
    from concourse.bass_utils import run_bass_kernel_spmd

    if "prog" not in _PROGRAM_CACHE:
        _PROGRAM_CACHE["prog"] = build_program()
    nc = _PROGRAM_CACHE["prog"]

    in_maps = stage_inputs(inputs)
    res = run_bass_kernel_spmd(nc, in_maps, core_ids=list(range(N_CORES)))
    out = np.concatenate([unpack_out(r["out"]) for r in res.results], axis=0)
    return out.astype(np.float32)
